# revision 1
# baseline (speedup 1.0000x reference)
"""Trainium2 Bass kernel for nn_BaselineGPT (sliding-window GQA attention block).

Sharding: 8 cores = 2 batches x 4 sequence chunks of 512 queries.
Each core computes its 512 output rows end-to-end (QKV proj, RMS norm, RoPE,
windowed GQA attention, output proj).  KV halo of 256 rows comes with the
chunk; chunk 0's missing halo is masked via a -30000 additive score bias
folded into an extra row of K^T.  Pair-head mixing is folded into Wo on the
host (it is linear and applied post-normalization).
"""

import math
from contextlib import ExitStack

import numpy as np

import concourse.bass as bass
from concourse import bacc
import concourse.mybir as mybir
import concourse.tile as tile
from concourse.masks import make_identity

B, S, DIM = 2, 2048, 1024
H, KVH, HD = 16, 4, 64
WINDOW = 256
ROPE_BASE = 10000.0
EPS = 1e-6

NQ = 512          # queries per core
NK = 768          # kv rows per core (incl 256 halo)
NCORES = 8
F32 = mybir.dt.float32
F32R = mybir.dt.float32r
BF16 = mybir.dt.bfloat16

_BUILT = None


def _build():
    nc = bacc.Bacc(None)

    xt = nc.declare_dram_parameter("xt", [DIM, NK], BF16, isOutput=False)
    wq = nc.declare_dram_parameter("wq", [DIM, DIM], BF16, isOutput=False)
    wk = nc.declare_dram_parameter("wk", [DIM, KVH * HD], BF16, isOutput=False)
    wv = nc.declare_dram_parameter("wv", [DIM, KVH * HD], BF16, isOutput=False)
    wo = nc.declare_dram_parameter("wo", [DIM, DIM], BF16, isOutput=False)
    cosk = nc.declare_dram_parameter("cosk", [NK, HD // 2], F32, isOutput=False)
    sink = nc.declare_dram_parameter("sink", [NK, HD // 2], F32, isOutput=False)
    kbias = nc.declare_dram_parameter("kbias", [1, NK], BF16, isOutput=False)
    qgain = nc.declare_dram_parameter("qgain", [1, H], F32, isOutput=False)
    m0 = nc.declare_dram_parameter("m0", [128, 512], BF16, isOutput=False)
    m2 = nc.declare_dram_parameter("m2", [128, 512], BF16, isOutput=False)
    out = nc.declare_dram_parameter("out", [NQ, DIM], F32, isOutput=True)

    with tile.TileContext(nc) as tc, ExitStack() as ctx:
        const = ctx.enter_context(tc.tile_pool(name="const", bufs=1))
        big = ctx.enter_context(tc.tile_pool(name="big", bufs=1))
        tmp = ctx.enter_context(tc.tile_pool(name="tmp", bufs=3))
        qtg_pool = ctx.enter_context(tc.tile_pool(name="qtg", bufs=5))
        att_pool = ctx.enter_context(tc.tile_pool(name="att", bufs=2))
        tn_pool = ctx.enter_context(tc.tile_pool(name="tn", bufs=2))
        outp = ctx.enter_context(tc.tile_pool(name="outp", bufs=2))
        ps_pool = ctx.enter_context(tc.tile_pool(name="ps", bufs=6, space="PSUM"))
        ps_bt = ctx.enter_context(tc.tile_pool(name="psbt", bufs=2, space="PSUM"))

        # ---- constants / small inputs ----
        ident = const.tile([128, 128], BF16, tag="ident")
        make_identity(nc, ident)
        ident_f32 = const.tile([128, 128], F32, tag="identf")
        make_identity(nc, ident_f32)
        eps_t = const.tile([128, 1], F32, tag="eps")
        nc.vector.memset(eps_t, EPS)
        ones64 = const.tile([1, 64], BF16, tag="ones64")
        nc.vector.memset(ones64, 1.0)
        qg_sb = const.tile([128, H], F32, tag="qg")
        nc.sync.dma_start(out=qg_sb, in_=qgain[0:1, :].to_broadcast((128, H)))
        m0_sb = const.tile([128, 512], BF16, tag="m0")
        nc.sync.dma_start(out=m0_sb, in_=m0[:, :])
        m2_sb = const.tile([128, 512], BF16, tag="m2")
        nc.sync.dma_start(out=m2_sb, in_=m2[:, :])
        cos_sb, sin_sb = [], []
        for st in range(6):
            sl = slice(st * 128, st * 128 + 128)
            tc_ = const.tile([128, HD // 2], F32, tag=f"cos{st}")
            nc.sync.dma_start(out=tc_, in_=cosk[sl, :])
            cos_sb.append(tc_)
            ts_ = const.tile([128, HD // 2], F32, tag=f"sin{st}")
            nc.sync.dma_start(out=ts_, in_=sink[sl, :])
            sin_sb.append(ts_)

        # ---- big persistent SBUF tensors ----
        xt_sb, wq_sb, wkv_sb = [], [], []
        for kt_ in range(8):
            sl = slice(kt_ * 128, kt_ * 128 + 128)
            for lst, nm, dram, w, eng in (
                (xt_sb, "xt", xt, NK, nc.sync),
                (wq_sb, "wq", wq, DIM, nc.scalar),
            ):
                t = big.tile([128, w], BF16, tag=f"{nm}{kt_}", name=f"{nm}{kt_}")
                eng.dma_start(out=t, in_=dram[sl, :])
                lst.append(t)
            t = big.tile([128, 512], BF16, tag=f"wkv{kt_}", name=f"wkv{kt_}")
            nc.gpsimd.dma_start(out=t[:, 0 : KVH * HD], in_=wk[sl, :])
            nc.gpsimd.dma_start(out=t[:, KVH * HD :], in_=wv[sl, :])
            wkv_sb.append(t)
        q_rope = big.tile([128, 4, DIM], BF16, tag="qrope")
        k_rope = big.tile([128, 6, KVH * HD], BF16, tag="krope")
        v_sb = big.tile([128, 6, KVH, HD + 1], BF16, tag="v")
        kt_sb = big.tile([128, KVH, NK], BF16, tag="kt")
        yt_sb = big.tile([128, 8, NQ], BF16, tag="yt")
        yraw = big.tile([64, 16, 512], BF16, tag="yraw")
        den_all = big.tile([16, 512], F32, tag="denall")
        rec_all = big.tile([16, 512], BF16, tag="recall")

        nc.vector.memset(v_sb[:, :, :, HD : HD + 1], 1.0)
        # bias row (row 64) of each k^T block, staged via SBUF to keep
        # kt_sb's writers on compute engines only (one semaphore)
        kb_sb = const.tile([1, NK], BF16, tag="kb")
        nc.sync.dma_start(out=kb_sb, in_=kbias[:, :])
        for g in range(KVH):
            nc.vector.tensor_copy(out=kt_sb[64:65, g, :], in_=kb_sb)

        def rmsnorm_rope(src_psum, nheads, st, dst, gain):
            """src_psum [128, nheads*HD] -> dst (slice of *_rope) with RMS norm,
            optional per-head gain (incl 1/8 scaling), and RoPE at kv tile st."""
            hw = nheads * HD
            sq = tmp.tile([128, 16, HD], F32, tag="sq")
            nc.scalar.activation(
                out=sq[:, :nheads, :],
                in_=src_psum.rearrange("p (h d) -> p h d", d=HD),
                func=mybir.ActivationFunctionType.Square,
            )
            ssq = tmp.tile([128, 16], F32, tag="ssq")
            nc.vector.tensor_reduce(
                out=ssq[:, :nheads],
                in_=sq[:, :nheads, :],
                axis=mybir.AxisListType.X,
                op=mybir.AluOpType.add,
            )
            # sqrt(mean + eps) then reciprocal (Rsqrt activation is banned)
            nc.scalar.activation(
                out=ssq[:, :nheads],
                in_=ssq[:, :nheads],
                func=mybir.ActivationFunctionType.Sqrt,
                bias=eps_t,
                scale=1.0 / HD,
            )
            inv = tmp.tile([128, 16], F32, tag="inv")
            nc.vector.reciprocal(out=inv[:, :nheads], in_=ssq[:, :nheads])
            if gain:
                nc.vector.tensor_mul(
                    out=inv[:, :nheads], in0=inv[:, :nheads], in1=qg_sb[:, :nheads]
                )
            invf = tmp.tile([128, 16, HD], F32, tag="invf")
            nc.vector.tensor_copy(
                out=invf[:, :nheads, :],
                in_=inv[:, :nheads].rearrange("p (h o) -> p h o", o=1).broadcast_to(
                    (128, nheads, HD)
                ),
            )
            rn = tmp.tile([128, 16, HD], F32, tag="rn")
            nc.vector.tensor_mul(
                out=rn[:, :nheads, :],
                in0=src_psum.rearrange("p (h d) -> p h d", d=HD),
                in1=invf[:, :nheads, :],
            )
            # RoPE: out1 = r1*cos + r2*sin ; out2 = r2*cos - r1*sin
            hd2 = HD // 2
            r1 = rn[:, :nheads, 0:hd2]
            r2 = rn[:, :nheads, hd2:HD]
            cosb = cos_sb[st].rearrange("p (o f) -> p o f", o=1).broadcast_to(
                (128, nheads, hd2)
            )
            sinb = sin_sb[st].rearrange("p (o f) -> p o f", o=1).broadcast_to(
                (128, nheads, hd2)
            )
            dd = dst.rearrange("p (h d) -> p h d", d=HD)
            o1 = dd[:, :, 0:hd2]
            o2 = dd[:, :, hd2:HD]
            t1 = tmp.tile([128, 16, hd2], F32, tag="ropet1")
            t2 = tmp.tile([128, 16, hd2], F32, tag="ropet2")
            nc.vector.tensor_mul(out=t1[:, :nheads, :], in0=r1, in1=cosb)
            nc.vector.tensor_mul(out=t2[:, :nheads, :], in0=r2, in1=sinb)
            nc.vector.tensor_add(out=o1, in0=t1[:, :nheads, :], in1=t2[:, :nheads, :])
            nc.vector.tensor_mul(out=t1[:, :nheads, :], in0=r2, in1=cosb)
            nc.vector.tensor_mul(out=t2[:, :nheads, :], in0=r1, in1=sinb)
            nc.vector.tensor_sub(out=o2, in0=t1[:, :nheads, :], in1=t2[:, :nheads, :])

        # ---- fused K|V projection over 6 kv s-tiles ----
        for st in range(6):
            pkv = ps_pool.tile([128, 512], F32, tag="p512")
            for kt_ in range(8):
                nc.tensor.matmul(
                    out=pkv,
                    lhsT=xt_sb[kt_][:, st * 128 : st * 128 + 128],
                    rhs=wkv_sb[kt_],
                    start=(kt_ == 0),
                    stop=(kt_ == 7),
                )
            nc.vector.tensor_copy(
                out=v_sb[:, st, :, 0:HD],
                in_=pkv[:, KVH * HD :].rearrange("p (g d) -> p g d", d=HD),
            )
            rmsnorm_rope(pkv[:, 0 : KVH * HD], KVH, st, k_rope[:, st, :], gain=False)

        # ---- Q projection over 4 q s-tiles (kv rows 256..768) ----
        for st in range(4):
            for half in range(2):
                pq = ps_pool.tile([128, 512], F32, tag="p512")
                for kt_ in range(8):
                    nc.tensor.matmul(
                        out=pq,
                        lhsT=xt_sb[kt_][:, 256 + st * 128 : 384 + st * 128],
                        rhs=wq_sb[kt_][:, half * 512 : half * 512 + 512],
                        start=(kt_ == 0),
                        stop=(kt_ == 7),
                    )
                rmsnorm_rope(
                    pq, 8, st + 2, q_rope[:, st, half * 512 : half * 512 + 512],
                    gain=True,
                )

        # wo reuses xt's SBUF slot (xt's last use is the Q projection above)
        wo_sb = []
        for kt_ in range(8):
            t = big.tile([128, DIM], BF16, tag=f"xt{kt_}", name=f"wo{kt_}")
            nc.sync.dma_start(out=t, in_=wo[kt_ * 128 : kt_ * 128 + 128, :])
            wo_sb.append(t)

        # ---- transpose K: k_rope [128s, (g,d)] -> kt_sb [d, g, s] ----
        for g in range(KVH):
            for half in range(2):
                ptk = ps_bt.tile([128, 512], BF16, tag="p512b")
                for i in range(3):
                    st = half * 3 + i
                    nc.tensor.transpose(
                        out=ptk[0:64, i * 128 : i * 128 + 128],
                        in_=k_rope[:, st, g * HD : g * HD + HD],
                        identity=ident,
                    )
                nc.vector.tensor_copy(
                    out=kt_sb[0:64, g, half * 384 : half * 384 + 384],
                    in_=ptk[0:64, 0:384],
                )

        # ---- per group: transpose Q tiles then attention over qblocks ----
        for g in range(KVH):
            qtg = {}
            for st in range(4):
                ptq = ps_bt.tile([128, 512], BF16, tag="p512b")
                for hh in range(4):
                    h = g * 4 + hh
                    nc.tensor.transpose(
                        out=ptq[0:64, hh * 128 : hh * 128 + 128],
                        in_=q_rope[:, st, h * HD : h * HD + HD],
                        identity=ident,
                    )
                qt = qtg_pool.tile([128, 512], BF16, tag="qtg")
                nc.vector.tensor_copy(out=qt[0:64, :], in_=ptq[0:64, :])
                nc.vector.memset(qt[64:65, :], 1.0)
                qtg[(g, st)] = qt

            for qb in range(4):
                att = att_pool.tile([128, 1536], BF16, tag="att")
                for t in range(3):
                    pss = ps_pool.tile([128, 512], F32, tag="p512")
                    nc.tensor.matmul(
                        out=pss,
                        lhsT=kt_sb[
                            0:65, g, qb * 128 + t * 128 : qb * 128 + t * 128 + 128
                        ],
                        rhs=qtg[(g, qb)][0:65, :],
                        start=True,
                        stop=True,
                    )
                    nc.scalar.activation(
                        out=att[:, t * 512 : t * 512 + 512],
                        in_=pss,
                        func=mybir.ActivationFunctionType.Exp,
                    )
                nc.vector.tensor_mul(out=att[:, 0:512], in0=att[:, 0:512], in1=m0_sb)
                nc.vector.tensor_mul(
                    out=att[:, 1024:1536], in0=att[:, 1024:1536], in1=m2_sb
                )
                psy = ps_pool.tile([128, 512], F32, tag="p512")
                for t in range(3):
                    nc.tensor.matmul(
                        out=psy[0:65, :],
                        lhsT=v_sb[:, qb + t, g, :],
                        rhs=att[:, t * 512 : t * 512 + 512],
                        start=(t == 0),
                        stop=(t == 2),
                    )
                it = g * 4 + qb
                nc.scalar.copy(out=yraw[:, it, :], in_=psy[0:64, :])
                dr = tn_pool.tile([1, 512], F32, tag="dr")
                nc.scalar.copy(out=dr, in_=psy[64:65, :])
                nc.gpsimd.dma_start(out=den_all[it : it + 1, :], in_=dr)

        # ---- batched softmax reciprocal: [16,512] -> [128,64] -> recip ----
        pden = ps_pool.tile([128, 512], F32, tag="p512")
        for c in range(4):
            nc.tensor.transpose(
                out=pden[:, c * 16 : c * 16 + 16],
                in_=den_all[:, c * 128 : c * 128 + 128],
                identity=ident_f32[0:16, 0:16],
            )
        rc = tn_pool.tile([128, 64], F32, tag="rc")
        nc.vector.reciprocal(out=rc, in_=pden[:, 0:64])
        prow = ps_pool.tile([128, 512], F32, tag="p512")
        for c in range(4):
            nc.tensor.transpose(
                out=prow[0:16, c * 128 : c * 128 + 128],
                in_=rc[:, c * 16 : c * 16 + 16],
                identity=ident_f32,
            )
        nc.scalar.copy(out=rec_all, in_=prow[0:16, :])

        # ---- normalize + scatter into yt pair layout ----
        for g in range(KVH):
            for qb in range(4):
                it = g * 4 + qb
                rrow = tn_pool.tile([1, 512], BF16, tag="rrow")
                nc.sync.dma_start(out=rrow, in_=rec_all[it : it + 1, :])
                rb = ps_pool.tile([128, 512], F32, tag="p512")
                nc.tensor.matmul(
                    out=rb[0:64, :], lhsT=ones64, rhs=rrow, start=True, stop=True
                )
                tn = tn_pool.tile([64, 512], BF16, tag="tnorm")
                nc.vector.tensor_mul(
                    out=tn, in0=yraw[:, it, :], in1=rb[0:64, :]
                )
                engs = [nc.gpsimd, nc.scalar, nc.sync, nc.gpsimd]
                for hh in range(4):
                    h = g * 4 + hh
                    pair, lo = h // 2, (h % 2) * 64
                    engs[hh].dma_start(
                        out=yt_sb[lo : lo + 64, pair, qb * 128 : qb * 128 + 128],
                        in_=tn[:, hh * 128 : hh * 128 + 128],
                    )

        # ---- output projection ----
        for qb in range(4):
            ob = outp.tile([128, DIM], F32, tag="ob")
            for half in range(2):
                po = ps_pool.tile([128, 512], F32, tag="p512")
                for p in range(8):
                    nc.tensor.matmul(
                        out=po,
                        lhsT=yt_sb[:, p, qb * 128 : qb * 128 + 128],
                        rhs=wo_sb[p][:, half * 512 : half * 512 + 512],
                        start=(p == 0),
                        stop=(p == 7),
                    )
                nc.scalar.copy(out=ob[:, half * 512 : half * 512 + 512], in_=po)
            nc.sync.dma_start(out=out[qb * 128 : qb * 128 + 128, :], in_=ob)

    nc.finalize()
    return nc


def _host_inputs(x, Wq, Wk, Wv, Wo, q_gain, pair_mix):
    """Build the 8 per-core input maps."""
    x = np.asarray(x, np.float32)
    Wq = np.asarray(Wq, np.float32)
    Wk = np.asarray(Wk, np.float32)
    Wv = np.asarray(Wv, np.float32)
    Wo = np.asarray(Wo, np.float32)
    q_gain = np.asarray(q_gain, np.float32)
    pair_mix = np.asarray(pair_mix, np.float32)

    # fold pair mixing into Wo:  out = y_mix @ Wo.T,  y_mix = y @ M.T  =>  Wo' = Wo @ M
    M = np.zeros((DIM, DIM), np.float32)
    eye = np.eye(HD, dtype=np.float32)
    for p in range(H // 2):
        for o in range(2):
            for i in range(2):
                ho, hi = 2 * p + o, 2 * p + i
                M[ho * HD : ho * HD + HD, hi * HD : hi * HD + HD] = (
                    pair_mix[p, o, i] * eye
                )
    woT = np.ascontiguousarray((Wo @ M).T)

    wqT = np.ascontiguousarray(Wq.T)
    wkT = np.ascontiguousarray(Wk.T)
    wvT = np.ascontiguousarray(Wv.T)
    qg8 = (q_gain / math.sqrt(HD)).reshape(1, H).astype(np.float32)

    inv_freq = 1.0 / (ROPE_BASE ** (np.arange(0, HD, 2, dtype=np.float32) / HD))

    ql = np.arange(128)
    m0_ = (ql[:, None] >= ql[None, :] + 1).astype(np.float32)  # kl >= ql+1
    m2_ = (ql[:, None] <= ql[None, :]).astype(np.float32)      # kl <= ql
    m0t = np.ascontiguousarray(np.tile(m0_, (1, 4)))
    m2t = np.ascontiguousarray(np.tile(m2_, (1, 4)))

    import ml_dtypes
    bf = ml_dtypes.bfloat16
    wqT, wkT, wvT, woT = (a.astype(bf) for a in (wqT, wkT, wvT, woT))
    m0t, m2t = m0t.astype(bf), m2t.astype(bf)
    in_maps = []
    for core in range(NCORES):
        b, c = core // 4, core % 4
        ks = 512 * c - 256
        xc = np.zeros((NK, DIM), np.float32)
        lo = max(0, ks)
        xc[lo - ks :] = x[b, lo : ks + NK]
        t = (ks + np.arange(NK, dtype=np.float32))[:, None]
        freqs = t * inv_freq[None, :]
        kb = np.where(t[:, 0] < 0, -30000.0, 0.0).astype(np.float32).reshape(1, NK)
        in_maps.append(
            {
                "xt": np.ascontiguousarray(xc.T).astype(bf),
                "wq": wqT,
                "wk": wkT,
                "wv": wvT,
                "wo": woT,
                "cosk": np.cos(freqs).astype(np.float32),
                "sink": np.sin(freqs).astype(np.float32),
                "kbias": kb.astype(bf),
                "qgain": qg8,
                "m0": m0t,
                "m2": m2t,
            }
        )
    return in_maps


def kernel(x, Wq, Wk, Wv, Wo, q_gain, pair_mix):
    global _BUILT
    from concourse.bass_utils import run_bass_kernel_spmd

    if _BUILT is None:
        _BUILT = _build()
    in_maps = _host_inputs(x, Wq, Wk, Wv, Wo, q_gain, pair_mix)
    res = run_bass_kernel_spmd(_BUILT, in_maps, list(range(NCORES)))
    out = np.empty((B, S, DIM), np.float32)
    for core in range(NCORES):
        b, c = core // 4, core % 4
        out[b, 512 * c : 512 * c + 512, :] = res.results[core]["out"]
    return out



# revision 9
# speedup vs baseline: 1.2150x; 1.2150x over previous
"""Trainium2 Bass kernel for nn_BaselineGPT (sliding-window GQA attention block).

Sharding: 8 cores = 2 batches x 4 sequence chunks of 512 queries.
Each core computes its 512 output rows end-to-end (QKV proj, RMS norm, RoPE,
windowed GQA attention, output proj).  KV halo of 256 rows comes with the
chunk; chunk 0's missing halo is masked via a -30000 bias folded into the
exp() activation's per-partition bias slot.  K-side rmsnorm is folded into
the exp() scale slot (rope commutes with per-head scaling).  Pair-head
mixing is folded into Wo on the host.

Perf notes: the PE p-state ramp means the tensor engine runs 2x faster when
continuously busy, so instruction order keeps the tensor queue dense; DMA
loads are split across the sync/scalar/gpsimd queues in dependency order so
the first matmul can start ~10us in.
"""

import math
from contextlib import ExitStack

import numpy as np

import concourse.bass as bass
from concourse import bacc
import concourse.mybir as mybir
import concourse.tile as tile
from concourse.masks import make_identity

B, S, DIM = 2, 2048, 1024
H, KVH, HD = 16, 4, 64
WINDOW = 256
ROPE_BASE = 10000.0
EPS = 1e-6

NQ = 512          # queries per core
NK = 768          # kv rows per core (incl 256 halo)
NCORES = 8
F32 = mybir.dt.float32
BF16 = mybir.dt.bfloat16

_BUILT = None


def _build():
    nc = bacc.Bacc(None)

    xt = nc.declare_dram_parameter("xt", [DIM, NK], BF16, isOutput=False)
    wq = nc.declare_dram_parameter("wq", [DIM, DIM], BF16, isOutput=False)
    wkv = nc.declare_dram_parameter("wkv", [DIM, 512], BF16, isOutput=False)
    wo = nc.declare_dram_parameter("wo", [DIM, DIM], BF16, isOutput=False)
    cs = nc.declare_dram_parameter("cs", [128, 6 * HD], F32, isOutput=False)
    kb = nc.declare_dram_parameter("kb", [128, 6], F32, isOutput=False)
    qg8 = nc.declare_dram_parameter("qg8", [128, H], F32, isOutput=False)
    m0 = nc.declare_dram_parameter("m0", [128, 512], BF16, isOutput=False)
    m2 = nc.declare_dram_parameter("m2", [128, 512], BF16, isOutput=False)
    sel = nc.declare_dram_parameter("sel", [4, 256], BF16, isOutput=False)
    out = nc.declare_dram_parameter("out", [NQ, DIM], BF16, isOutput=True)

    with tile.TileContext(nc) as tc, ExitStack() as ctx:
        const = ctx.enter_context(tc.tile_pool(name="const", bufs=1))
        big = ctx.enter_context(tc.tile_pool(name="big", bufs=1))
        tmp = ctx.enter_context(tc.tile_pool(name="tmp", bufs=3))
        att_pool = ctx.enter_context(tc.tile_pool(name="att", bufs=3))
        ysb_pool = ctx.enter_context(tc.tile_pool(name="ysb", bufs=6))
        ob_pool = ctx.enter_context(tc.tile_pool(name="ob", bufs=2))
        ps_proj = ctx.enter_context(tc.tile_pool(name="psp", bufs=3, space="PSUM"))
        ps_pss = ctx.enter_context(tc.tile_pool(name="pss", bufs=3, space="PSUM"))
        ps_y = ctx.enter_context(tc.tile_pool(name="psy", bufs=2, space="PSUM"))

        # ---- preload DMAs, ordered by first use across the 3 dma queues ----
        xt_sb = [None] * 8
        wkv_sb = [None] * 8
        wq_sb = [None] * 8
        wo_sb = [None] * 8
        for kt_ in range(8):
            xt_sb[kt_] = big.tile([128, NK], BF16, tag=f"xt{kt_}", name=f"xt{kt_}")
            wkv_sb[kt_] = big.tile([128, 512], BF16, tag=f"wkv{kt_}", name=f"wkv{kt_}")
            wq_sb[kt_] = big.tile([128, DIM], BF16, tag=f"wq{kt_}", name=f"wq{kt_}")
            wo_sb[kt_] = big.tile([128, DIM], BF16, tag=f"wo{kt_}", name=f"wo{kt_}")
        cs_sb = const.tile([128, 6, HD], F32, tag="cs")
        kb_sb = const.tile([128, 6], F32, tag="kb")
        qg_sb = const.tile([128, H], F32, tag="qg")
        m0_sb = const.tile([128, 512], BF16, tag="m0")
        m2_sb = const.tile([128, 512], BF16, tag="m2")
        sel_sb = const.tile([4, 256], BF16, tag="sel")

        def dram_tile(t, dram, kt_):
            return (t, dram[kt_ * 128: kt_ * 128 + 128, :])

        # interleave so the kt=0..7 accumulation chain unblocks in order
        sync_q = [dram_tile(wkv_sb[0], wkv, 0), dram_tile(xt_sb[0], xt, 0),
                  dram_tile(xt_sb[1], xt, 1), dram_tile(xt_sb[2], xt, 2),
                  dram_tile(wkv_sb[1], wkv, 1), dram_tile(wkv_sb[2], wkv, 2),
                  dram_tile(wq_sb[0], wq, 0), dram_tile(wq_sb[1], wq, 1),
                  dram_tile(wq_sb[2], wq, 2), dram_tile(wq_sb[3], wq, 3),
                  (m0_sb, m0[:, :]),
                  dram_tile(wo_sb[0], wo, 0), dram_tile(wo_sb[1], wo, 1),
                  dram_tile(wo_sb[2], wo, 2), dram_tile(wo_sb[3], wo, 3)]
        scal_q = [dram_tile(xt_sb[3], xt, 3), dram_tile(xt_sb[4], xt, 4),
                  dram_tile(xt_sb[5], xt, 5), dram_tile(wkv_sb[3], wkv, 3),
                  dram_tile(wkv_sb[4], wkv, 4),
                  dram_tile(wq_sb[4], wq, 4), dram_tile(wq_sb[5], wq, 5),
                  dram_tile(wq_sb[6], wq, 6), dram_tile(wq_sb[7], wq, 7),
                  (m2_sb, m2[:, :]),
                  dram_tile(wo_sb[4], wo, 4), dram_tile(wo_sb[5], wo, 5),
                  dram_tile(wo_sb[6], wo, 6), dram_tile(wo_sb[7], wo, 7)]
        gps_q = [dram_tile(xt_sb[6], xt, 6), dram_tile(xt_sb[7], xt, 7),
                 (cs_sb.rearrange("p a b -> p (a b)"), cs[:, :]),
                 dram_tile(wkv_sb[5], wkv, 5), dram_tile(wkv_sb[6], wkv, 6),
                 dram_tile(wkv_sb[7], wkv, 7),
                 (kb_sb, kb[:, :]), (qg_sb, qg8[:, :]),
                 (sel_sb, sel[:, :])]
        for t, src in sync_q:
            nc.sync.dma_start(out=t, in_=src)
        for t, src in scal_q:
            nc.scalar.dma_start(out=t, in_=src)
        for t, src in gps_q:
            nc.gpsimd.dma_start(out=t, in_=src)

        # ---- constants ----
        ident = const.tile([128, 128], BF16, tag="ident")
        make_identity(nc, ident)
        eps_t = const.tile([128, 1], F32, tag="eps")
        nc.vector.memset(eps_t, EPS)
        ones64 = const.tile([1, 64], BF16, tag="ones64")
        nc.vector.memset(ones64, 1.0)
        ident_f32 = const.tile([128, 128], F32, tag="identf")
        make_identity(nc, ident_f32)

        # ---- persistent SBUF tensors ----
        k_rope = big.tile([128, 6, KVH * HD], BF16, tag="krope")
        q_rope = big.tile([128, 4, DIM], BF16, tag="qrope")
        v_sb = big.tile([128, 6, KVH, HD + 1], BF16, tag="v")
        kt_sb = big.tile([64, KVH, NK], BF16, tag="kt")
        qt_sb = big.tile([64, 16, 512], BF16, tag="qt")
        yt_sb = big.tile([128, 8, 512], BF16, tag="yt")
        ssqk = big.tile([128, 6, KVH], F32, tag="ssqk")
        invk = big.tile([128, 6, KVH], F32, tag="invk")
        den_all = big.tile([4, 512], BF16, tag="den")
        rec4_sb = big.tile([4, 512], BF16, tag="rec4")
        nc.vector.memset(v_sb[:, :, :, HD:HD + 1], 1.0)

        def rope(eng, dst, src, nh, st, tmp_tag):
            """dst[:, h, 0:32] = r1*cos + r2*sin ; dst[:, h, 32:64] = r2*cos - r1*sin"""
            hd2 = HD // 2
            r1 = src[:, :, 0:hd2]
            r2 = src[:, :, hd2:HD]
            cosb = cs_sb[:, st, 0:hd2].rearrange("p (o f) -> p o f", o=1).broadcast_to(
                (128, nh, hd2))
            sinb = cs_sb[:, st, hd2:HD].rearrange("p (o f) -> p o f", o=1).broadcast_to(
                (128, nh, hd2))
            t1 = tmp.tile([128, nh, hd2], BF16, tag=tmp_tag)
            t2 = tmp.tile([128, nh, hd2], BF16, tag=tmp_tag + "b")
            eng.tensor_mul(out=t1, in0=r1, in1=cosb)
            eng.tensor_mul(out=t2, in0=r2, in1=sinb)
            eng.tensor_add(out=dst[:, :, 0:hd2], in0=t1, in1=t2)
            eng.tensor_mul(out=t1, in0=r2, in1=cosb)
            eng.tensor_mul(out=t2, in0=r1, in1=sinb)
            eng.tensor_sub(out=dst[:, :, hd2:HD], in0=t1, in1=t2)

        # ---- fused KV (+Q) projection over the 6 kv s-tiles ----
        for st in range(6):
            pkv = ps_proj.tile([128, 512], F32, tag="pp")
            for kt_ in range(8):
                nc.tensor.matmul(
                    out=pkv,
                    lhsT=xt_sb[kt_][:, st * 128: st * 128 + 128],
                    rhs=wkv_sb[kt_],
                    start=(kt_ == 0),
                    stop=(kt_ == 7),
                )
            # K: rope (raw; norm folded into exp scale), V: copy, sumsq of k_rope
            kraw = tmp.tile([128, KVH, HD], BF16, tag="kraw")
            nc.scalar.copy(
                out=kraw, in_=pkv[:, 0:KVH * HD].rearrange("p (g d) -> p g d", d=HD))
            kr = k_rope[:, st, :].rearrange("p (g d) -> p g d", d=HD)
            rope(nc.gpsimd, kr, kraw, KVH, st, "kr")
            nc.scalar.copy(
                out=v_sb[:, st, :, 0:HD],
                in_=pkv[:, KVH * HD:].rearrange("p (g d) -> p g d", d=HD),
            )
            sqk = tmp.tile([128, KVH, HD], F32, tag="sqk")
            nc.gpsimd.tensor_mul(out=sqk, in0=kr, in1=kr)
            nc.vector.tensor_reduce(
                out=ssqk[:, st, :], in_=sqk,
                axis=mybir.AxisListType.X, op=mybir.AluOpType.add)
            # K transpose: k_rope [128s, (g d)] -> kt_sb [d, g, 128s-block]
            ptk = ps_pss.tile([128, 512], BF16, tag="ps")
            for g in range(KVH):
                nc.tensor.transpose(
                    out=ptk[0:HD, g * 128: g * 128 + 128],
                    in_=k_rope[:, st, g * HD: g * HD + HD],
                    identity=ident,
                )
            nc.vector.tensor_copy(
                out=kt_sb[:, :, st * 128: st * 128 + 128],
                in_=ptk[0:HD, :].rearrange("p (g s) -> p g s", s=128),
            )

            if st >= 2:
                qst = st - 2
                qraw = tmp.tile([128, H, HD], BF16, tag="qraw")
                for half in range(2):
                    pq = ps_proj.tile([128, 512], F32, tag="pp")
                    for kt_ in range(8):
                        nc.tensor.matmul(
                            out=pq,
                            lhsT=xt_sb[kt_][:, st * 128: st * 128 + 128],
                            rhs=wq_sb[kt_][:, half * 512: half * 512 + 512],
                            start=(kt_ == 0),
                            stop=(kt_ == 7),
                        )
                    nc.scalar.copy(
                        out=qraw[:, half * 8: half * 8 + 8, :],
                        in_=pq.rearrange("p (h d) -> p h d", d=HD),
                    )
                qr = q_rope[:, qst, :].rearrange("p (h d) -> p h d", d=HD)
                rope(nc.vector, qr, qraw, H, st, "qr")
                # rms norm: sumsq on post-rope q (rotation preserves norms)
                sqq = tmp.tile([128, H, HD], BF16, tag="sqq")
                ssqq = tmp.tile([128, H], F32, tag="ssqq")
                nc.gpsimd.tensor_mul(out=sqq, in0=qr, in1=qr)
                nc.vector.tensor_reduce(
                    out=ssqq, in_=sqq, axis=mybir.AxisListType.X,
                    op=mybir.AluOpType.add)
                nc.scalar.activation(
                    out=ssqq, in_=ssqq, func=mybir.ActivationFunctionType.Sqrt,
                    bias=eps_t, scale=1.0 / HD)
                invq = tmp.tile([128, H], F32, tag="invq")
                nc.vector.reciprocal(out=invq, in_=ssqq)
                nc.vector.tensor_mul(out=invq, in0=invq, in1=qg_sb)
                nc.vector.tensor_mul(
                    out=qr, in0=qr,
                    in1=invq.rearrange("p (h o) -> p h o", o=1).broadcast_to(
                        (128, H, HD)))
                # Q transpose per group -> qt_sb[:, g*4+qst, :]
                for g in range(KVH):
                    ptq = ps_pss.tile([128, 512], BF16, tag="ps")
                    for hh in range(4):
                        h = g * 4 + hh
                        nc.tensor.transpose(
                            out=ptq[0:HD, hh * 128: hh * 128 + 128],
                            in_=q_rope[:, qst, h * HD: h * HD + HD],
                            identity=ident,
                        )
                    nc.scalar.copy(
                        out=qt_sb[:, g * 4 + qst, :], in_=ptq[0:HD, :])

        # K-side rms inv, batched: invk = 1/sqrt(mean+eps), folded into exp scale
        nc.scalar.activation(
            out=ssqk, in_=ssqk, func=mybir.ActivationFunctionType.Sqrt,
            bias=eps_t, scale=1.0 / HD)
        nc.vector.reciprocal(out=invk, in_=ssqk)

        # ---- attention, qb-major with software pipelining ----
        ysbs = {}

        def emit_scores(qb, g):
            att = att_pool.tile([128, 3, 512], BF16, tag="att")
            for t in range(3):
                pss = ps_pss.tile([128, 512], F32, tag="ps")
                nc.tensor.matmul(
                    out=pss,
                    lhsT=kt_sb[:, g, qb * 128 + t * 128: qb * 128 + t * 128 + 128],
                    rhs=qt_sb[:, g * 4 + qb, :],
                    start=True, stop=True,
                )
                nc.scalar.activation(
                    out=att[:, t, :], in_=pss,
                    func=mybir.ActivationFunctionType.Exp,
                    bias=kb_sb[:, qb + t: qb + t + 1],
                    scale=invk[:, qb + t, g: g + 1],
                )
            nc.gpsimd.tensor_mul(out=att[:, 0, :], in0=att[:, 0, :], in1=m0_sb)
            nc.gpsimd.tensor_mul(out=att[:, 2, :], in0=att[:, 2, :], in1=m2_sb)
            return att

        def emit_attv(qb, g, att):
            psy = ps_y.tile([128, 512], F32, tag="py")
            for t in range(3):
                nc.tensor.matmul(
                    out=psy[0:HD + 1, :],
                    lhsT=v_sb[:, qb + t, g, :],
                    rhs=att[:, t, :],
                    start=(t == 0), stop=(t == 2),
                )
            ysb = ysb_pool.tile([HD + 1, 512], BF16, tag="ysb")
            nc.vector.tensor_copy(out=ysb, in_=psy[0:HD + 1, :])
            nc.gpsimd.dma_start(out=den_all[g: g + 1, :], in_=ysb[64:65, :])
            return ysb

        def emit_outproj(qb):
            ob = ob_pool.tile([128, DIM], BF16, tag="ob")
            for half in range(2):
                po = ps_pss.tile([128, 512], F32, tag="ps")
                for p in range(8):
                    nc.tensor.matmul(
                        out=po,
                        lhsT=yt_sb[:, p, qb * 128: qb * 128 + 128],
                        rhs=wo_sb[p][:, half * 512: half * 512 + 512],
                        start=(p == 0), stop=(p == 7),
                    )
                nc.scalar.copy(out=ob[:, half * 512: half * 512 + 512], in_=po)
            nc.sync.dma_start(out=out[qb * 128: qb * 128 + 128, :], in_=ob)

        for qb in range(4):
            atts = {}
            for g in range(KVH):
                atts[g] = emit_scores(qb, g)
                if g >= 1:
                    ysbs[g - 1] = emit_attv(qb, g - 1, atts[g - 1])
            ysbs[3] = emit_attv(qb, 3, atts[3])
            if qb >= 1:
                emit_outproj(qb - 1)
            # batched softmax denominator reciprocal for this qb
            pd = ps_pss.tile([128, 512], BF16, tag="ps")
            for i in range(4):
                nc.tensor.transpose(
                    out=pd[:, i * 4: i * 4 + 4],
                    in_=den_all[:, i * 128: i * 128 + 128],
                    identity=ident[0:4, 0:4],
                )
            rc = tmp.tile([128, 16], F32, tag="rc")
            nc.vector.reciprocal(out=rc, in_=pd[:, 0:16])
            pr = ps_pss.tile([128, 512], F32, tag="ps")
            for i in range(4):
                nc.tensor.transpose(
                    out=pr[0:4, i * 128: i * 128 + 128],
                    in_=rc[:, i * 4: i * 4 + 4],
                    identity=ident_f32,
                )
            nc.scalar.copy(out=rec4_sb, in_=pr[0:4, :])
            # normalize + scatter into yt pair layout
            for g in range(KVH):
                prb = ps_pss.tile([128, 512], F32, tag="ps")
                nc.tensor.matmul(
                    out=prb[0:HD, :], lhsT=sel_sb[:, g * 64: g * 64 + 64],
                    rhs=rec4_sb, start=True, stop=True)
                ysb4 = ysbs[g].rearrange("p (h s) -> p h s", s=128)
                prb4 = prb[0:HD, :].rearrange("p (h s) -> p h s", s=128)
                for lo in range(2):
                    nc.vector.tensor_mul(
                        out=yt_sb[lo * 64: lo * 64 + 64, 2 * g: 2 * g + 2,
                                  qb * 128: qb * 128 + 128],
                        in0=ysb4[0:HD, 2 * lo: 2 * lo + 2, :],
                        in1=prb4[:, 2 * lo: 2 * lo + 2, :],
                    )
        emit_outproj(3)

    nc.finalize()
    return nc


def _host_inputs(x, Wq, Wk, Wv, Wo, q_gain, pair_mix):
    """Build the 8 per-core input maps."""
    x = np.asarray(x, np.float32)
    Wq = np.asarray(Wq, np.float32)
    Wk = np.asarray(Wk, np.float32)
    Wv = np.asarray(Wv, np.float32)
    Wo = np.asarray(Wo, np.float32)
    q_gain = np.asarray(q_gain, np.float32)
    pair_mix = np.asarray(pair_mix, np.float32)

    # fold pair mixing into Wo:  out = y_mix @ Wo.T,  y_mix = y @ M.T  =>  Wo' = Wo @ M
    M = np.zeros((DIM, DIM), np.float32)
    eye = np.eye(HD, dtype=np.float32)
    for p in range(H // 2):
        for o in range(2):
            for i in range(2):
                ho, hi = 2 * p + o, 2 * p + i
                M[ho * HD: ho * HD + HD, hi * HD: hi * HD + HD] = (
                    pair_mix[p, o, i] * eye
                )
    woT = np.ascontiguousarray((Wo @ M).T)  # [in=(h,d), out]
    # permute rows into the yt pair layout: row blk*128 + lo*64 + d
    # holds head h = 4*(blk//2) + 2*lo + blk%2, dim d
    perm = np.empty(DIM, np.int64)
    for blk in range(8):
        for lo in range(2):
            h = 4 * (blk // 2) + 2 * lo + (blk % 2)
            perm[blk * 128 + lo * 64: blk * 128 + lo * 64 + HD] = (
                np.arange(HD) + h * HD)
    woT = woT[perm]

    wqT = np.ascontiguousarray(Wq.T)
    wkvT = np.ascontiguousarray(np.concatenate([Wk.T, Wv.T], axis=1))
    qg8 = np.tile((q_gain / math.sqrt(HD)).reshape(1, H), (128, 1)).astype(np.float32)

    inv_freq = 1.0 / (ROPE_BASE ** (np.arange(0, HD, 2, dtype=np.float32) / HD))

    ql = np.arange(128)
    m0_ = (ql[:, None] >= ql[None, :] + 1).astype(np.float32)  # kl >= ql+1
    m2_ = (ql[:, None] <= ql[None, :]).astype(np.float32)      # kl <= ql
    m0t = np.ascontiguousarray(np.tile(m0_, (1, 4)))
    m2t = np.ascontiguousarray(np.tile(m2_, (1, 4)))

    import ml_dtypes
    bf = ml_dtypes.bfloat16
    wqT, wkvT, woT = (a.astype(bf) for a in (wqT, wkvT, woT))
    m0t, m2t = m0t.astype(bf), m2t.astype(bf)
    sel_np = np.zeros((4, 256), np.float32)
    for g in range(4):
        sel_np[g, g * 64:(g + 1) * 64] = 1.0
    sel_np = sel_np.astype(bf)
    in_maps = []
    for core in range(NCORES):
        b, c = core // 4, core % 4
        ks = 512 * c - 256
        xc = np.zeros((NK, DIM), np.float32)
        lo = max(0, ks)
        xc[lo - ks:] = x[b, lo: ks + NK]
        pos = ks + np.arange(NK, dtype=np.float32)
        freqs = pos[:, None] * inv_freq[None, :]        # [NK, 32]
        # cs[p, st*64 + j]: j<32 cos, j>=32 sin, for kv row st*128+p
        csk = np.concatenate([np.cos(freqs), np.sin(freqs)], axis=1)  # [NK, 64]
        csk = np.ascontiguousarray(
            csk.reshape(6, 128, HD).transpose(1, 0, 2).reshape(128, 6 * HD))
        kbias = np.where(pos < 0, -30000.0, 0.0).astype(np.float32)
        kbias = np.ascontiguousarray(kbias.reshape(6, 128).T)  # [128, 6]
        in_maps.append(
            {
                "xt": np.ascontiguousarray(xc.T).astype(bf),
                "wq": wqT,
                "wkv": wkvT,
                "wo": woT,
                "cs": csk,
                "kb": kbias,
                "qg8": qg8,
                "m0": m0t,
                "m2": m2t,
                "sel": sel_np,
            }
        )
    return in_maps


def kernel(x, Wq, Wk, Wv, Wo, q_gain, pair_mix):
    global _BUILT
    from concourse.bass_utils import run_bass_kernel_spmd

    if _BUILT is None:
        _BUILT = _build()
    in_maps = _host_inputs(x, Wq, Wk, Wv, Wo, q_gain, pair_mix)
    res = run_bass_kernel_spmd(_BUILT, in_maps, list(range(NCORES)))
    out = np.empty((B, S, DIM), np.float32)
    for core in range(NCORES):
        b, c = core // 4, core % 4
        out[b, 512 * c: 512 * c + 512, :] = res.results[core]["out"].astype(np.float32)
    return out
